# revision 2
# baseline (speedup 1.0000x reference)
"""Trainium2 Bass kernel for nn_BDH_90984587198975 (6-layer BDH with Hebbian
fast weights), SPMD over 8 NeuronCores.

Sharding: tensor-parallel over the flattened latent dim NHL=4*8192.  Core c
owns a 4096-wide slice of head h=c//2 (half=c%2), with lanes permuted so rope
pairs split into [even-members(2048) | odd-members(2048)] (rotation becomes a
tile swap instead of a cross-partition shuffle).  F (fast weights) stays
sharded by latent rows - its update is local.

v2 changes vs baseline:
- wenc/wencv/wdec persistent in SBUF (loaded once at start via batched DMAs)
  instead of re-streamed every layer: removes ~480us of SP-engine descriptor
  issue and 36MB of HBM traffic.
- PSUM: two pools, mm(2 banks) for transient tiles, sQ(3)/sP(3) rings whose
  tag lifetimes permit cross-layer overlap.
- P5/P6/P7 and both AllReduces split by batch element b: AR(b=0) overlaps
  compute for b=1, and P7/make_xT/next-layer P1 for b=0 overlap AR(b=1),
  hiding most of the ~67us/layer PE-idle at the layer boundary.
- Hebbian: chunk transposes evicted on Scalar engine, F-update fused to
  [128,512] ops (halves those DVE instructions).
"""
import math
import numpy as np
import ml_dtypes

import concourse.bass as bass
import concourse.mybir as mybir
import concourse.tile as tile
from concourse import bacc
from concourse.masks import make_identity
from concourse.bass_utils import run_bass_kernel_spmd

BF = ml_dtypes.bfloat16
f32 = mybir.dt.float32
bf16 = mybir.dt.bfloat16
AF = mybir.ActivationFunctionType
OP = mybir.AluOpType

N_LAYER = 6
D = 256
NH = 4
VOCAB = 130
LR = 0.01
L = 8192
EPS = 1e-5
TWO_PI = 2.0 * math.pi
THETA = 65536.0
B, T = 2, 512
NCORE = 8
SH = 4096          # latent shard per core
NLT = SH // 128    # 32 latent tiles
NBT = (B * T) // 128  # 8 bt tiles
CHK = 128          # hebbian time chunk
NCH = T // CHK     # 4 chunks

_CACHE = {}

# scsb strip offsets: ut strip widths 512,384,256,128
SOFF = [0, 512, 896, 1152]


# ----------------------------------------------------------------- builder --
def _emit(nc, n_layer, fake_cc=False):
    # ---- DRAM I/O ----
    oh = nc.dram_tensor("onehotT", [256, 1024], bf16, kind="ExternalInput")
    emb = nc.dram_tensor("embedp", [256, 256], bf16, kind="ExternalInput")
    wenc = nc.dram_tensor("wenc", [NLT * 256, 128], bf16, kind="ExternalInput")
    wencv = nc.dram_tensor("wencv", [NLT * 256, 128], bf16, kind="ExternalInput")
    wdec = nc.dram_tensor("wdec", [SH, 256], bf16, kind="ExternalInput")
    freqs = nc.dram_tensor("freqst", [128, 16], f32, kind="ExternalInput")
    lmh = nc.dram_tensor("lmh", [256, 130], bf16, kind="ExternalInput")
    out = nc.dram_tensor("out", [1024, 130], bf16, kind="ExternalOutput")

    from contextlib import ExitStack
    tc = tile.TileContext(nc)
    with tc, ExitStack() as stk:
        per = stk.enter_context(tc.tile_pool(name="per", bufs=1))
        sm = stk.enter_context(tc.tile_pool(name="sm", bufs=2))
        qp = stk.enter_context(tc.tile_pool(name="qp", bufs=3))
        ysp = stk.enter_context(tc.tile_pool(name="ysp", bufs=3))
        tp = stk.enter_context(tc.tile_pool(name="tp", bufs=3))
        mm = stk.enter_context(tc.tile_pool(name="mm", bufs=2, space="PSUM"))
        sc = stk.enter_context(tc.tile_pool(name="sc", bufs=3, space="PSUM"))
        dram = stk.enter_context(tc.tile_pool(name="dram", bufs=2, space="DRAM"))

        # ---- persistent tiles ----
        xs = per.tile([128, NLT * 1024], bf16, tag="xs")
        F = per.tile([128, NLT * 256], bf16, tag="F")
        xf = per.tile([128, 2048], f32, tag="xf")
        xbf = per.tile([128, 2048], bf16, tag="xbf")
        xTbf = per.tile([128, 2048], bf16, tag="xTbf")
        ymlp = per.tile([128, 2048], bf16, tag="ymlp")
        attn = per.tile([128, 2048], bf16, tag="attn")
        ykv = per.tile([128, 2048], bf16, tag="ykv")
        ykvT = per.tile([128, 2048], bf16, tag="ykvT")
        wesb = per.tile([128, NLT * 256], bf16, tag="wesb")
        wvsb = per.tile([128, NLT * 256], bf16, tag="wvsb")
        wdsb = per.tile([128, NLT * 256], bf16, tag="wdsb")
        ident = per.tile([128, 128], bf16, tag="ident")
        maskU = per.tile([128, 128], f32, tag="maskU")
        maskS = per.tile([128, 128], f32, tag="maskS")
        maskS0 = per.tile([128, 128], f32, tag="maskS0")
        epst = per.tile([128, 1], f32, tag="epst")
        xsh = [per.tile([128, 256], bf16, tag=f"xsh{c}", name=f"xsh{c}") for c in range(B)]
        xbt = [per.tile([128, SH], bf16, tag=f"xbt{c}", name=f"xbt{c}") for c in range(B)]
        scsb = [per.tile([128, 1280], bf16, tag=f"scsb{b}", name=f"scsb{b}") for b in range(B)]
        ssb = [per.tile([128, 256], bf16, tag=f"ssb{b}", name=f"ssb{b}") for b in range(B)]

        freqsb = per.tile([128, 16], f32, tag="freqsb")
        trigd = dram.tile([2048, 1024], bf16, tag="trigd", bufs=1)
        nc.sync.dma_start(freqsb[:], freqs[:])

        make_identity(nc, ident[:])
        nc.vector.memset(epst[:], EPS)
        # masks generated on-device: maskU[u,t] = (t>u); maskS = LR*(u<t);
        # maskS0 = LR*((u+1)<t) with row 127 zeroed
        nc.gpsimd.memset(maskU[:], 1.0)
        nc.gpsimd.affine_select(out=maskU[:], in_=maskU[:], compare_op=OP.is_gt,
                                fill=0.0, base=0, pattern=[[1, 128]],
                                channel_multiplier=-1)
        nc.gpsimd.memset(maskS[:], LR)
        nc.gpsimd.affine_select(out=maskS[:], in_=maskS[:], compare_op=OP.is_gt,
                                fill=0.0, base=0, pattern=[[1, 128]],
                                channel_multiplier=-1)
        nc.gpsimd.memset(maskS0[:], LR)
        nc.gpsimd.affine_select(out=maskS0[:], in_=maskS0[:], compare_op=OP.is_gt,
                                fill=0.0, base=-1, pattern=[[1, 128]],
                                channel_multiplier=-1)
        nc.gpsimd.affine_select(out=maskS0[:], in_=maskS0[:],
                                compare_op=OP.not_equal, fill=0.0, base=-127,
                                pattern=[[0, 128]], channel_multiplier=1)
        nc.vector.memset(F[:], 0.0)
        for c2 in range(B):
            nc.vector.memset(xbt[c2][96:128, :], 0.0)
            nc.vector.memset(xsh[c2][96:128, :], 0.0)

        # embedding inputs first (needed immediately; weights not until P1)
        ohsb = [sm.tile([128, 1024], bf16, tag=f"oh{v}", bufs=1, name=f"oh{v}") for v in range(2)]
        embsb = [sm.tile([128, 256], bf16, tag=f"em{v}", bufs=1, name=f"em{v}") for v in range(2)]
        for v in range(2):
            nc.sync.dma_start(ohsb[v][:], oh[v * 128:(v + 1) * 128, :])
            nc.sync.dma_start(embsb[v][:], emb[v * 128:(v + 1) * 128, :])

        # persistent weights, loaded once in 4-lt (1024-row / 512-row) batches
        for g in range(NLT // 4):  # 8 batches
            for wsb, wdr, s in ((wesb, wenc, 8), (wvsb, wencv, 8)):
                nc.sync.dma_start(
                    wsb[:, g * 1024:(g + 1) * 1024].rearrange(
                        "p (s c) -> p s c", s=s),
                    wdr[g * 1024:(g + 1) * 1024, :].rearrange(
                        "(s p) c -> p s c", p=128))
            nc.sync.dma_start(
                wdsb[:, g * 1024:(g + 1) * 1024].rearrange(
                    "p (s c) -> p s c", s=4),
                wdec[g * 512:(g + 1) * 512, :].rearrange(
                    "(s p) c -> p s c", p=128))

        # on-device rotary table: trigd[pt*128+p, 0:512]=cos, 512:1024=sin.
        # y = t*freq (turns); k = round(y) via the f32 +-2^23 trick;
        # rad = 2pi*(y-k) in [-pi,pi]; sin/cos via Sin table (+pi/2 wrap).
        R23 = 8388608.0
        for pt in range(16):
            gph = sm.tile([128, 512], f32, tag="gph", bufs=1)
            gw = sm.tile([128, 512], f32, tag="gw", bufs=1)
            tg = tp.tile([128, 1024], bf16, tag="trig")
            nc.vector.memset(gw[:], 1.0)
            nc.vector.tensor_tensor_scan(gph[:], gw[:], gw[:], -1.0,
                                         op0=OP.add, op1=OP.bypass)
            nc.vector.tensor_scalar(gph[:], gph[:], freqsb[:, pt:pt + 1], None,
                                    op0=OP.mult)
            nc.vector.tensor_scalar(gw[:], gph[:], R23, R23,
                                    op0=OP.add, op1=OP.subtract)
            nc.vector.tensor_tensor(gph[:], gph[:], gw[:], op=OP.subtract)
            nc.vector.tensor_scalar(gph[:], gph[:], TWO_PI, None, op0=OP.mult)
            nc.scalar.activation(tg[:, 512:1024], gph[:], AF.Sin)
            nc.vector.add_range_wrap(gw[:], gph[:], shift=math.pi / 2.0,
                                     bound=math.pi, period=TWO_PI)
            nc.scalar.activation(tg[:, 0:512], gw[:], AF.Sin)
            nc.sync.dma_start(trigd[pt * 128:(pt + 1) * 128, :], tg[:])

        # ---------------- helpers ----------------
        def ln_stats(src_ap):
            ssum = sm.tile([128, 1], f32, tag="ssum", bufs=2)
            nmean = sm.tile([128, 1], f32, tag="nmean", bufs=2)
            cent = sm.tile([128, 256], f32, tag="cent", bufs=2)
            sq = sm.tile([128, 256], f32, tag="sq", bufs=2)
            sqsum = sm.tile([128, 1], f32, tag="sqsum", bufs=2)
            std = sm.tile([128, 1], f32, tag="std", bufs=2)
            rstd = sm.tile([128, 1], f32, tag="rstd", bufs=2)
            nc.vector.tensor_reduce(ssum[:], src_ap, axis=mybir.AxisListType.X,
                                    op=OP.add)
            nc.vector.tensor_scalar_mul(nmean[:], ssum[:], -1.0 / 256.0)
            nc.scalar.activation(cent[:], src_ap, AF.Identity, bias=nmean[:])
            nc.scalar.activation(sq[:], cent[:], AF.Square, accum_out=sqsum[:])
            nc.scalar.activation(std[:], sqsum[:], AF.Sqrt, scale=1.0 / 256.0,
                                 bias=epst[:])
            nc.vector.reciprocal(rstd[:], std[:])
            return cent, rstd

        def emit_ln(src_ap, outs):
            cent, rstd = ln_stats(src_ap)
            for oap in outs:
                nc.scalar.activation(oap, cent[:], AF.Copy, scale=rstd[:])

        def make_xT(b):
            """xTbf[dh*1024 + (b*4+jj)*128 + t] = xbf^T for this b half."""
            for dh in range(2):
                pt = mm.tile([128, 512], bf16, tag="mm", name="ptT")
                for jj in range(4):
                    j = b * 4 + jj
                    nc.tensor.transpose(
                        pt[:, jj * 128:jj * 128 + 128],
                        xbf[:, j * 256 + dh * 128:j * 256 + dh * 128 + 128],
                        ident[:])
                dst = xTbf[:, dh * 1024 + b * 512:dh * 1024 + b * 512 + 512]
                if dh == 0:
                    nc.vector.tensor_copy(dst, pt[:])
                else:
                    nc.scalar.copy(dst, pt[:])

        # ---------------- embedding ----------------
        for j in range(NBT):
            pe = mm.tile([128, 512], f32, tag="mm", name="pe")
            for v in range(2):
                nc.tensor.matmul(pe[:, 0:256], ohsb[v][:, j * 128:(j + 1) * 128],
                                 embsb[v][:], start=(v == 0), stop=(v == 1))
            emit_ln(pe[:, 0:256],
                    [xf[:, j * 256:(j + 1) * 256], xbf[:, j * 256:(j + 1) * 256]])
        for b in range(B):
            make_xT(b)

        # ---------------- layers ----------------
        ar_a = {}
        ar_y = {}
        for lay in range(n_layer):
            # -- P1 per b --
            for b in range(B):
                # P1: xs^T = relu(wenc^T x) for this b's bt half
                for lt in range(NLT):
                    pp = mm.tile([128, 512], f32, tag="mm", name="pp")
                    for dh in range(2):
                        nc.tensor.matmul(
                            pp[:], wesb[:, lt * 256 + dh * 128:lt * 256 + dh * 128 + 128],
                            xTbf[:, dh * 1024 + b * 512:dh * 1024 + b * 512 + 512],
                            start=(dh == 0), stop=(dh == 1))
                    dst = xs[:, lt * 1024 + b * 512:lt * 1024 + b * 512 + 512]
                    if lt % 2 == 0:
                        nc.scalar.activation(dst, pp[:], AF.Relu)
                    else:
                        nc.vector.tensor_scalar_max(dst, pp[:], 0.0)

            # -- P2: one trig pass for both b -> 6 score strips --
            strips = {}
            for b in range(B):
                sA = sc.tile([128, 512], f32, tag="big", bufs=6, name=f"sA{b}")
                sB = sc.tile([128, 512], f32, tag="big", bufs=6, name=f"sB{b}")
                sC = sc.tile([128, 512], f32, tag="big", bufs=6, name=f"sC{b}")
                strips[b] = {0: (sA, 0, 512), 1: (sB, 0, 384),
                             2: (sC, 0, 256), 3: (sB, 384, 128)}
            for pt in range(16):
                tg = tp.tile([128, 1024], bf16, tag="trig")
                nc.sync.dma_start(tg[:], trigd[pt * 128:(pt + 1) * 128, :])
                ct, st = tg[:, 0:512], tg[:, 512:1024]
                for b in range(B):
                    xe = xs[:, pt * 1024 + b * 512:pt * 1024 + b * 512 + 512]
                    xo = xs[:, (16 + pt) * 1024 + b * 512:(16 + pt) * 1024 + b * 512 + 512]
                    t1 = sm.tile([128, 512], bf16, tag="ropet1", bufs=1)
                    t2 = sm.tile([128, 512], bf16, tag="ropet2", bufs=1)
                    t3 = sm.tile([128, 512], bf16, tag="ropet3", bufs=1)
                    t4 = sm.tile([128, 512], bf16, tag="ropet4", bufs=1)
                    qe = qp.tile([128, 512], bf16, tag="q")
                    qo = qp.tile([128, 512], bf16, tag="q")
                    nc.vector.tensor_tensor(t1[:], xe, ct, op=OP.mult)
                    nc.vector.tensor_tensor(t2[:], xo, st, op=OP.mult)
                    nc.vector.tensor_tensor(qe[:], t1[:], t2[:], op=OP.subtract)
                    nc.gpsimd.tensor_tensor(t3[:], xo, ct, op=OP.mult)
                    nc.gpsimd.tensor_tensor(t4[:], xe, st, op=OP.mult)
                    nc.gpsimd.tensor_tensor(qo[:], t3[:], t4[:], op=OP.add)
                    for qi, qt in enumerate((qe, qo)):
                        lt = pt if qi == 0 else 16 + pt
                        for ut in range(4):
                            stile, soff, n = strips[b][ut]
                            nc.tensor.matmul(
                                stile[:, soff:soff + n],
                                qt[:, ut * 128:(ut + 1) * 128],
                                qt[:, ut * 128:512],
                                start=(lt == 0), stop=(lt == 31))

            # -- evict scores + P3 + pair-AR per b --
            for b in range(B):
                for ut in range(4):
                    stile, soff, n = strips[b][ut]
                    o = SOFF[ut]
                    nc.vector.tensor_tensor(scsb[b][:, o:o + 128],
                                            stile[:, soff:soff + 128],
                                            maskU[:], op=OP.mult)
                    if n > 128:
                        nc.scalar.copy(scsb[b][:, o + 128:o + n],
                                       stile[:, soff + 128:soff + n])

                # P3: attn half per t-tile
                for tt in range(4):
                    pa = sc.tile([128, 512], f32, tag="big", bufs=6, name="pa")
                    for ut in range(tt + 1):
                        lhs = scsb[b][:, SOFF[ut] + (tt - ut) * 128:SOFF[ut] + (tt - ut) * 128 + 128]
                        rhs = xbf[:, (b * 4 + ut) * 256:(b * 4 + ut) * 256 + 256]
                        nc.tensor.matmul(pa[:, 0:256], lhs, rhs, start=(ut == 0),
                                         stop=(ut == tt))
                    j = b * 4 + tt
                    nc.scalar.copy(attn[:, j * 256:(j + 1) * 256], pa[:, 0:256])

                # pair AllReduce of this b's attn half
                a_src = dram.tile([512, 256], bf16, tag=f"asrc{b}", bufs=1)
                a_dst = dram.tile([512, 256], bf16, tag=f"adst{b}", bufs=1)
                for h in range(2):
                    nc.sync.dma_start(
                        a_src[h * 256:(h + 1) * 256, :].rearrange(
                            "(s p) c -> p s c", p=128),
                        attn[:, (b * 4 + h * 2) * 256:(b * 4 + h * 2 + 2) * 256
                             ].rearrange("p (s c) -> p s c", s=2))
                if fake_cc:
                    nc.gpsimd.dma_start(a_dst[:], a_src[:])
                else:
                    nc.gpsimd.collective_compute(
                        "AllReduce", OP.add,
                        replica_groups=[[0, 1], [2, 3], [4, 5], [6, 7]],
                        ins=[a_src.opt()], outs=[a_dst.opt()])
                ar_a[b] = (a_src, a_dst)

            # -- P4: Hebbian chunks (overlaps the ARs) --
            for c2 in range(B):
                nc.sync.dma_start(xsh[c2][0:127, :],
                                  xbf[1:128, (c2 * 4) * 256:(c2 * 4) * 256 + 256])
            for k in range(NCH):
                # transposes: xbt[c2] partition p = xs time (k*128-1+p)
                for c2 in range(B):
                    for lt4 in range(NLT // 4):
                        pt = mm.tile([128, 512], bf16, tag="mm", name="pt4")
                        for q4 in range(4):
                            lt = lt4 * 4 + q4
                            base = lt * 1024 + c2 * 512 + k * 128 - 1
                            if k == 0:
                                src = xs[:, lt * 1024 + c2 * 512:lt * 1024 + c2 * 512 + 127]
                                nc.tensor.transpose(pt[0:127, q4 * 128:q4 * 128 + 128],
                                                    src, ident[:])
                            else:
                                nc.tensor.transpose(pt[:, q4 * 128:q4 * 128 + 128],
                                                    xs[:, base:base + 128], ident[:])
                        rows = slice(0, 127) if k == 0 else slice(0, 128)
                        nc.scalar.copy(xbt[c2][rows, lt4 * 512:lt4 * 512 + 512],
                                       pt[rows, :])
                mS = maskS0 if k == 0 else maskS
                for b in range(B):
                    # S^T blocks [u,t] for both c2 in one tile (first, so the
                    # ssb eviction chain hides under the H-term matmuls)
                    pst = sc.tile([128, 512], f32, tag="big", bufs=6, name="pst")
                    for c2 in range(B):
                        for lt in range(NLT):
                            if k == 0:
                                lhs = xs[:, lt * 1024 + c2 * 512:lt * 1024 + c2 * 512 + 127]
                                m = 127
                            else:
                                base = lt * 1024 + c2 * 512 + k * 128 - 1
                                lhs = xs[:, base:base + 128]
                                m = 128
                            nc.tensor.matmul(
                                pst[0:m, c2 * 128:c2 * 128 + 128], lhs,
                                xs[:, lt * 1024 + b * 512 + k * 128:lt * 1024 + b * 512 + k * 128 + 128],
                                start=(lt == 0), stop=(lt == NLT - 1))
                    if k == 0:
                        nc.vector.memset(ssb[b][96:128, :], 0.0)
                    rws = slice(0, 127) if k == 0 else slice(0, 128)
                    for c2 in range(B):
                        nc.vector.tensor_tensor(ssb[b][rws, c2 * 128:c2 * 128 + 128],
                                                pst[rws, c2 * 128:c2 * 128 + 128],
                                                mS[rws, :], op=OP.mult)
                    # H term + S intra into one psum tile
                    ph = sc.tile([128, 512], f32, tag="big", bufs=6, name="ph")
                    for lt in range(NLT):
                        nc.tensor.matmul(
                            ph[:, 0:256],
                            xs[:, lt * 1024 + b * 512 + k * 128:lt * 1024 + b * 512 + k * 128 + 128],
                            F[:, lt * 256:(lt + 1) * 256],
                            start=(lt == 0), stop=False)
                    # S apply into same psum as H
                    for c2 in range(B):
                        rhs = (xsh[c2][:] if k == 0
                               else xbf[:, (c2 * 4 + k) * 256:(c2 * 4 + k) * 256 + 256])
                        nc.tensor.matmul(ph[:, 0:256], ssb[b][:, c2 * 128:c2 * 128 + 128],
                                         rhs, start=False, stop=(c2 == B - 1))
                    j = b * 4 + k
                    nc.vector.tensor_copy(ymlp[:, j * 256:(j + 1) * 256], ph[:, 0:256])
                # dF (2-lt pairs) and fused F update
                for lth in range(NLT // 2):
                    pdf = mm.tile([128, 512], f32, tag="mm", name="pdf")
                    for half in range(2):
                        lt = lth * 2 + half
                        for c2 in range(B):
                            rhs = (xsh[c2][:] if k == 0
                                   else xbf[:, (c2 * 4 + k) * 256:(c2 * 4 + k) * 256 + 256])
                            nc.tensor.matmul(pdf[:, half * 256:half * 256 + 256],
                                             xbt[c2][:, lt * 128:(lt + 1) * 128], rhs,
                                             start=(c2 == 0), stop=(c2 == B - 1))
                    nc.vector.scalar_tensor_tensor(
                        F[:, lth * 512:(lth + 1) * 512], pdf[:], LR,
                        F[:, lth * 512:(lth + 1) * 512], op0=OP.mult, op1=OP.add)

            # -- P5 + P6 + AR_y per b --
            for b in range(B):
                # P5: attn AR result -> LN -> y_kv -> y_kv^T
                a_src, a_dst = ar_a[b]
                for h in range(2):
                    nc.sync.dma_start(
                        attn[:, (b * 4 + h * 2) * 256:(b * 4 + h * 2 + 2) * 256
                             ].rearrange("p (s c) -> p s c", s=2),
                        a_dst[h * 256:(h + 1) * 256, :].rearrange(
                            "(s p) c -> p s c", p=128))
                for jj in range(4):
                    j = b * 4 + jj
                    emit_ln(attn[:, j * 256:(j + 1) * 256],
                            [ykv[:, j * 256:(j + 1) * 256]])
                for dh in range(2):
                    ptk = mm.tile([128, 512], bf16, tag="mm", name="ptk")
                    for jj in range(4):
                        j = b * 4 + jj
                        nc.tensor.transpose(
                            ptk[:, jj * 128:jj * 128 + 128],
                            ykv[:, j * 256 + dh * 128:j * 256 + dh * 128 + 128],
                            ident[:])
                    dst = ykvT[:, dh * 1024 + b * 512:dh * 1024 + b * 512 + 512]
                    if dh == 0:
                        nc.vector.tensor_copy(dst, ptk[:])
                    else:
                        nc.scalar.copy(dst, ptk[:])

                # P6: ys, xy, decoder accumulation for this b
                pdec = [sc.tile([128, 512], f32, tag="big", bufs=6,
                                name=f"pdec{i}") for i in range(2)]
                for lt in range(NLT):
                    ys = ysp.tile([128, 512], bf16, tag="ys")
                    pp2 = mm.tile([128, 512], f32, tag="mm", name="pp2")
                    for dh in range(2):
                        nc.tensor.matmul(
                            pp2[:], wvsb[:, lt * 256 + dh * 128:lt * 256 + dh * 128 + 128],
                            ykvT[:, dh * 1024 + b * 512:dh * 1024 + b * 512 + 512],
                            start=(dh == 0), stop=(dh == 1))
                    if lt % 2 == 0:
                        nc.scalar.activation(ys[:], pp2[:], AF.Relu)
                    else:
                        nc.vector.tensor_scalar_max(ys[:], pp2[:], 0.0)
                    nc.gpsimd.tensor_tensor(ys[:], ys[:],
                                            xs[:, lt * 1024 + b * 512:lt * 1024 + b * 512 + 512],
                                            op=OP.mult)
                    for jj in range(4):
                        nc.tensor.matmul(pdec[jj // 2][:, (jj % 2) * 256:(jj % 2) * 256 + 256],
                                         ys[:, jj * 128:(jj + 1) * 128],
                                         wdsb[:, lt * 256:(lt + 1) * 256],
                                         start=(lt == 0), stop=(lt == NLT - 1))
                # finalize ymlp half, AR
                for i in range(2):
                    o = (b * 4 + i * 2) * 256
                    nc.vector.tensor_tensor(ymlp[:, o:o + 512], pdec[i][:],
                                            ymlp[:, o:o + 512], op=OP.add)
                y_src = dram.tile([512, 256], bf16, tag=f"ysrc{b}", bufs=1)
                y_dst = dram.tile([512, 256], bf16, tag=f"ydst{b}", bufs=1)
                for h in range(2):
                    nc.sync.dma_start(
                        y_src[h * 256:(h + 1) * 256, :].rearrange(
                            "(s p) c -> p s c", p=128),
                        ymlp[:, (b * 4 + h * 2) * 256:(b * 4 + h * 2 + 2) * 256
                             ].rearrange("p (s c) -> p s c", s=2))
                if fake_cc:
                    nc.gpsimd.dma_start(y_dst[:], y_src[:])
                else:
                    nc.gpsimd.collective_compute(
                        "AllReduce", OP.add, replica_groups=[list(range(NCORE))],
                        ins=[y_src.opt()], outs=[y_dst.opt()])
                ar_y[b] = (y_src, y_dst)

            # -- P7 + make_xT per b (b=0 overlaps AR_y(b=1)) --
            for b in range(B):
                y_src, y_dst = ar_y[b]
                for h in range(2):
                    nc.sync.dma_start(
                        ymlp[:, (b * 4 + h * 2) * 256:(b * 4 + h * 2 + 2) * 256
                             ].rearrange("p (s c) -> p s c", s=2),
                        y_dst[h * 256:(h + 1) * 256, :].rearrange(
                            "(s p) c -> p s c", p=128))
                for jj in range(4):
                    j = b * 4 + jj
                    lny = sm.tile([128, 256], f32, tag="lny", bufs=1)
                    emit_ln(ymlp[:, j * 256:(j + 1) * 256], [lny[:]])
                    z = sm.tile([128, 256], f32, tag="z", bufs=1)
                    nc.vector.tensor_tensor(z[:], lny[:], xf[:, j * 256:(j + 1) * 256],
                                            op=OP.add)
                    emit_ln(z[:], [xf[:, j * 256:(j + 1) * 256],
                                   xbf[:, j * 256:(j + 1) * 256]])
                make_xT(b)

        # ---------------- lm head ----------------
        lsb = sm.tile([128, 260], bf16, tag="lmh", bufs=1)
        for dh in range(2):
            nc.sync.dma_start(lsb[:, dh * 130:(dh + 1) * 130],
                              lmh[dh * 128:(dh + 1) * 128, :])
        for j in range(NBT):
            pl = mm.tile([128, 512], f32, tag="mm", name="pl")
            for dh in range(2):
                nc.tensor.matmul(pl[:, 0:130],
                                 xTbf[:, dh * 1024 + j * 128:dh * 1024 + j * 128 + 128],
                                 lsb[:, dh * 130:(dh + 1) * 130],
                                 start=(dh == 0), stop=(dh == 1))
            lg = sm.tile([128, 130], bf16, tag="lg")
            nc.scalar.copy(lg[:], pl[:, 0:130])
            nc.sync.dma_start(out[j * 128:(j + 1) * 128, :], lg[:])

    return nc


def _get_nc(n_layer=N_LAYER, fake_cc=False):
    key = (n_layer, fake_cc)
    if key not in _CACHE:
        nc = bacc.Bacc("TRN2", target_bir_lowering=False, debug=False,
                       num_devices=NCORE, dynamic_dma_scratch_size=2048)
        _emit(nc, n_layer, fake_cc)
        nc.compile()
        _CACHE[key] = nc
    return _CACHE[key]


# -------------------------------------------------------------- host side --
def _perm_local():
    p = np.empty(SH, np.int64)
    p[:2048] = 2 * np.arange(2048)
    p[2048:] = 2 * np.arange(2048) + 1
    return p


def host_prep(idx, embed_w, encoder, encoder_v, decoder, lm_head):
    idx = np.asarray(idx).astype(np.int64)
    embed_w = np.asarray(embed_w, np.float32)
    encoder = np.asarray(encoder, np.float32)
    encoder_v = np.asarray(encoder_v, np.float32)
    decoder = np.asarray(decoder, np.float32)
    lm_head = np.asarray(lm_head, np.float32)
    perm = _perm_local()

    onehotT = np.zeros((256, 1024), np.float32)
    flat = idx.reshape(-1)
    onehotT[flat, np.arange(1024)] = 1.0
    embedp = np.zeros((256, 256), np.float32)
    embedp[:VOCAB] = embed_w

    lmh = lm_head.astype(BF)

    in_maps = []
    for c in range(NCORE):
        h, half = c // 2, c % 2
        g = half * SH + perm
        we = encoder[h][:, g]            # [256, 4096]
        wv = encoder_v[h][:, g]
        dec = decoder[h * L + g, :]      # [4096, 256]
        wet = np.ascontiguousarray(
            we.reshape(2, 128, NLT, 128).transpose(2, 0, 1, 3).reshape(NLT * 256, 128)
        ).astype(BF)
        wvt = np.ascontiguousarray(
            wv.reshape(2, 128, NLT, 128).transpose(2, 0, 1, 3).reshape(NLT * 256, 128)
        ).astype(BF)
        pg = half * 2048 + np.arange(2048)
        freqt = ((1.0 / (THETA ** ((2.0 * pg) / L))) / TWO_PI).astype(np.float32)
        # freqst[p, pt] = freq-in-turns[pt*128 + p]
        f2 = np.ascontiguousarray(freqt.reshape(16, 128).T)
        in_maps.append({
            "onehotT": onehotT.astype(BF), "embedp": embedp.astype(BF),
            "wenc": wet, "wencv": wvt,
            "wdec": np.ascontiguousarray(dec).astype(BF),
            "freqst": f2, "lmh": lmh,
        })
    return in_maps


def kernel(idx, embed_w, encoder, encoder_v, decoder, lm_head,
           n_layer=N_LAYER, _return_raw=False):
    in_maps = host_prep(idx, embed_w, encoder, encoder_v, decoder, lm_head)
    nc = _get_nc(n_layer)
    r = run_bass_kernel_spmd(nc, in_maps, core_ids=list(range(NCORE)))
    if _return_raw:
        return r
    return np.ascontiguousarray(
        r.results[0]["out"].reshape(B, T, VOCAB).astype(np.float32))
